# revision 2
# baseline (speedup 1.0000x reference)
"""Trainium2 Bass kernel: batched causal single-head self-attention.

Reference computation (per batch b):
    q = x @ Wq; k = x @ Wk; v = x @ Wv          # [T, H] each, contraction over E
    S = (q @ k^T) / sqrt(H)                     # [T, T]
    P = softmax(causal_mask(S), axis=-1)
    out = P @ v                                 # [T, H]

Shapes: x [512, 256, 384] f32, W* [384, 64] f32, out [512, 256, 64] f32.
Sharding: pure data parallel, 64 batches per NeuronCore across 8 cores.

Device algorithm per batch pair (all matmul operands bf16, fp32 PSUM accum):
  - host ships xT = x^T per batch ([E, T] layout, E on partitions).
  - qk^T = [Wq|Wk]^T @ xT        (one packed 128-wide stationary, 3 E-chunks,
                                  both batches of the pair in one N=512 MM)
  - v    = xT^T-chunk @ Wv       (x chunks stationary, Wv moving -> v in
                                  [t, h] layout directly; no PE transpose)
  - S^T  = k^T.T @ q^T           ([tk, tq] layout; lower-left T/4 block skipped)
  - P    = exp(0.125 * S^T)      (ScalarE; no max-subtraction needed, |s|<~45)
  - P   *= causal 0/1 tril       (only the two diagonal blocks)
  - out_aug[tq, 0:65] = sum_tk P[tk,tq] * [v|1][tk]  (col 64 = softmax denom)
  - out_aug -> SBUF bf16 -> HBM; the denominator divide happens on HOST.
The per-quad loop is software-pipelined: projections for quad q are emitted
before attention for quad q-1 so the PE never waits on the copy->shift chain.
"""

import numpy as np
import ml_dtypes

B, T, E, H = 512, 256, 384, 64
NCORES = 8
BPC = B // NCORES  # 64
P = 128
EC = E // P  # 3
HP1 = H + 1  # 65

_cache: dict = {}


def _install_ntff_hook():
    """Shim antenv.axon_hooks (absent in this image) so run_bass_kernel_spmd
    trace=True can capture NTFF profiles via the axon .so's C ABI."""
    import contextlib
    import ctypes
    import sys
    import types

    if "antenv.axon_hooks" in sys.modules:
        return
    so_path = "/opt/axon/libaxon_pjrt.so"
    lib = ctypes.CDLL(so_path)
    if not hasattr(lib, "axon_start_nrt_profile"):
        return
    lib.axon_start_nrt_profile.argtypes = [
        ctypes.POINTER(ctypes.c_int64),
        ctypes.c_size_t,
    ]
    lib.axon_start_nrt_profile.restype = ctypes.c_int64
    lib.axon_stop_nrt_profile.argtypes = [ctypes.c_char_p]
    lib.axon_stop_nrt_profile.restype = ctypes.c_int64

    @contextlib.contextmanager
    def _hook(output_dir, device_ids):
        import jax

        jax.devices()
        if device_ids:
            ids = (ctypes.c_int64 * len(device_ids))(*device_ids)
            rc = lib.axon_start_nrt_profile(ids, len(device_ids))
        else:
            rc = lib.axon_start_nrt_profile(None, 0)
        if rc != 0:
            raise RuntimeError(f"axon_start_nrt_profile rc={rc}")
        try:
            yield
        finally:
            n = lib.axon_stop_nrt_profile(str(output_dir).encode())
            if n < 0:
                raise RuntimeError(f"axon_stop_nrt_profile rc={n}")
            print(f"profile: {n} file(s) written to {output_dir}", file=sys.stderr)

    mod = types.ModuleType("antenv.axon_hooks")
    _state = {"hook": _hook}
    mod.get_axon_ntff_profile_hook = lambda: _state["hook"]
    mod.set_axon_ntff_profile_hook = lambda h: _state.__setitem__("hook", h)
    sys.modules["antenv.axon_hooks"] = mod


def _build_program(bpc):
    import concourse.bacc as bacc
    import concourse.mybir as mybir
    import concourse.tile as tile

    f32 = mybir.dt.float32
    bf16 = mybir.dt.bfloat16
    Exp = mybir.ActivationFunctionType.Exp
    Mult = mybir.AluOpType.mult

    nc = bacc.Bacc(
        "TRN2",
        target_bir_lowering=False,
        debug=False,
        enable_asserts=False,
        num_devices=NCORES,
    )
    xt_d = nc.dram_tensor("xt", [bpc, P, EC, T], bf16, kind="ExternalInput").ap()
    wqk_d = nc.dram_tensor("wqk", [P, EC, P], bf16, kind="ExternalInput").ap()
    wv_d = nc.dram_tensor("wv", [P, EC, H], bf16, kind="ExternalInput").ap()
    # 0/1 tril mask (tk <= tq) for the two diagonal 128x128 score blocks
    um_d = nc.dram_tensor("um", [P, P], bf16, kind="ExternalInput").ap()
    out_d = nc.dram_tensor("out", [bpc, T, HP1], bf16, kind="ExternalOutput").ap()

    Q = 4
    assert bpc % Q == 0
    nquads = bpc // Q

    with tile.TileContext(nc) as tc:
        with (
            tc.tile_pool(name="const", bufs=1) as constp,
            tc.tile_pool(name="xin", bufs=3) as xpool,
            tc.tile_pool(name="qksb", bufs=3) as qkpool,
            tc.tile_pool(name="psb", bufs=3) as ppool,
            tc.tile_pool(name="osb", bufs=3) as opool,
            tc.tile_pool(name="ps_qk", bufs=2, space="PSUM") as ps_qk,
            tc.tile_pool(name="ps_v", bufs=2, space="PSUM") as ps_v,
            tc.tile_pool(name="ps_s", bufs=2, space="PSUM") as ps_s,
            tc.tile_pool(name="ps_o", bufs=2, space="PSUM") as ps_o,
        ):
            wqk = constp.tile([P, EC, P], bf16)
            nc.sync.dma_start(wqk, wqk_d)
            wv = constp.tile([P, EC, H], bf16)
            nc.sync.dma_start(wv, wv_d)
            um = constp.tile([P, P], bf16)
            nc.sync.dma_start(um, um_d)
            # v staging [tk, s, j, h|1] with a persistent ones column at h=64
            # (manual rotation so the ones column survives across iterations)
            v_augs = []
            for i in range(4):
                va = constp.tile([P, 2, 2, HP1], bf16, name=f"vaug{i}")
                nc.vector.memset(va[:, :, :, H : H + 1], 1.0)
                v_augs.append(va)
            # k^T staging padded to 128 partitions with zero rows 64:128 so the
            # scores matmuls use full-width stationaries and stream q^T
            # directly from qk_sb (zero k rows null out the garbage rows)
            kabs = []
            for i in range(2):
                kt = constp.tile([P, Q, T], bf16, name=f"kab{i}")
                nc.vector.memset(kt[H:P], 0.0)
                kabs.append(kt)

            state = {}

            def emit_proj(qd):
                b0 = Q * qd
                xt = xpool.tile([P, Q, EC, T], bf16)
                nc.sync.dma_start(
                    xt, xt_d[b0 : b0 + Q].rearrange("s p c t -> p s c t")
                )
                qk_sb = qkpool.tile([P, Q, T], bf16)
                k_sb = kabs[qd % 2]
                for prl in range(Q // 2):
                    s0 = 2 * prl
                    pr = qd * (Q // 2) + prl
                    qk_ps = ps_qk.tile([P, 2, T], f32)
                    for c in range(EC):
                        nc.tensor.matmul(
                            qk_ps,
                            wqk[:, c, :],
                            xt[:, s0 : s0 + 2, c, :],
                            start=(c == 0),
                            stop=(c == EC - 1),
                        )
                    v_ps = ps_v.tile([P, 2, 2, H], f32)
                    for s in range(2):
                        for j in range(2):
                            for c in range(EC):
                                nc.tensor.matmul(
                                    v_ps[:, s, j, :],
                                    xt[:, s0 + s, c, j * P : (j + 1) * P],
                                    wv[:, c, :],
                                    start=(c == 0),
                                    stop=(c == EC - 1),
                                )
                    # PSUM -> SBUF bf16 casts
                    nc.vector.tensor_copy(qk_sb[:, s0 : s0 + 2, :], qk_ps)
                    v_aug = v_augs[pr % 4]
                    nc.vector.tensor_copy(v_aug[:, :, :, 0:H], v_ps)
                # k^T partitions 64:128 -> 0:64 (DMA shift), whole quad at once
                nc.sync.dma_start(k_sb[0:H], qk_sb[H:P])
                state[qd] = (qk_sb, k_sb)

            def emit_attn(qd):
                b0 = Q * qd
                qk_sb, k_sb = state.pop(qd)
                for prl in range(Q // 2):
                    s0 = 2 * prl
                    pr = qd * (Q // 2) + prl
                    v_aug = v_augs[pr % 4]
                    p_sb = ppool.tile([P, 2, EC, P], bf16)
                    for s in range(2):
                        s_ps = ps_s.tile([P, EC * P], f32, name="s_ps")
                        # S^T[tk 0:128, tq 0:256]
                        nc.tensor.matmul(
                            s_ps[:, 0:T],
                            k_sb[:, s0 + s, 0:P],
                            qk_sb[:, s0 + s, :],
                            start=True,
                            stop=True,
                        )
                        # S^T[tk 128:256, tq 128:256]
                        nc.tensor.matmul(
                            s_ps[:, T : 3 * P],
                            k_sb[:, s0 + s, P:T],
                            qk_sb[:, s0 + s, P:T],
                            start=True,
                            stop=True,
                        )
                        nc.scalar.activation(
                            p_sb[:, s], s_ps, Exp, scale=0.125
                        )

                    # multiplicative causal mask, diagonal blocks only
                    for blk in (0, 2):
                        nc.vector.tensor_tensor(
                            p_sb[:, :, blk, :],
                            p_sb[:, :, blk, :],
                            um[:, None, :].to_broadcast([P, 2, P]),
                            Mult,
                        )

                    o_ps = ps_o.tile([P, 2, 2, HP1], f32)
                    for s in range(2):
                        nc.tensor.matmul(
                            o_ps[:, s, 0, :],
                            p_sb[:, s, 0, :],
                            v_aug[:, s, 0, :],
                            start=True,
                            stop=True,
                        )
                        nc.tensor.matmul(
                            o_ps[:, s, 1, :],
                            p_sb[:, s, 1, :],
                            v_aug[:, s, 0, :],
                            start=True,
                            stop=False,
                        )
                        nc.tensor.matmul(
                            o_ps[:, s, 1, :],
                            p_sb[:, s, 2, :],
                            v_aug[:, s, 1, :],
                            start=False,
                            stop=True,
                        )

                    o_sb = opool.tile([P, 2, 2, HP1], bf16)
                    nc.scalar.copy(o_sb, o_ps)
                    nc.gpsimd.dma_start(
                        out_d[b0 + s0 : b0 + s0 + 2].rearrange(
                            "s (j p) h -> p s j h", p=P
                        ),
                        o_sb,
                    )

            for qd in range(nquads):
                emit_proj(qd)
                if qd > 0:
                    emit_attn(qd - 1)
            emit_attn(nquads - 1)

    nc.compile()
    return nc


def _prep_inputs(x, Wq, Wk, Wv, bpc):
    bf = ml_dtypes.bfloat16
    nb = NCORES * bpc
    x = np.asarray(x, dtype=np.float32)[:nb]
    # [b, t, e] -> [b, p, c, t] with e = c*128 + p
    xt = np.ascontiguousarray(
        x.reshape(nb, T, EC, P).transpose(0, 3, 2, 1)
    ).astype(bf)
    wqk = np.concatenate(
        [np.asarray(Wq, np.float32), np.asarray(Wk, np.float32)], axis=1
    )  # [E, 128]
    wqk = np.ascontiguousarray(wqk.reshape(EC, P, P).transpose(1, 0, 2)).astype(bf)
    wv = np.ascontiguousarray(
        np.asarray(Wv, np.float32).reshape(EC, P, H).transpose(1, 0, 2)
    ).astype(bf)
    um = (np.arange(P)[:, None] <= np.arange(P)[None, :]).astype(np.float32).astype(bf)
    per_core = []
    for c in range(NCORES):
        per_core.append(
            {
                "xt": xt[c * bpc : (c + 1) * bpc],
                "wqk": wqk,
                "wv": wv,
                "um": um,
            }
        )
    return per_core


def kernel(x, Wq, Wk, Wv, _trace=False, _bpc=BPC):
    """Full inputs in, full output out. Shards batch dim over 8 NeuronCores."""
    from concourse import bass_utils

    if _trace:
        _install_ntff_hook()

    key = ("prog", _bpc)
    if key not in _cache:
        _cache[key] = _build_program(_bpc)
    nc = _cache[key]

    in_maps = _prep_inputs(x, Wq, Wk, Wv, _bpc)
    res = bass_utils.run_bass_kernel_spmd(
        nc, in_maps, core_ids=list(range(NCORES)), trace=_trace
    )
    _cache["last_result"] = res
    aug = np.concatenate(
        [np.asarray(r["out"]) for r in res.results], axis=0
    ).astype(np.float32)
    out = aug[:, :, 0:H] / aug[:, :, H : H + 1]
    return out


# revision 5
# speedup vs baseline: 1.0121x; 1.0121x over previous
"""Trainium2 Bass kernel: batched causal single-head self-attention.

Reference computation (per batch b):
    q = x @ Wq; k = x @ Wk; v = x @ Wv          # [T, H] each, contraction over E
    S = (q @ k^T) / sqrt(H)                     # [T, T]
    P = softmax(causal_mask(S), axis=-1)
    out = P @ v                                 # [T, H]

Shapes: x [512, 256, 384] f32, W* [384, 64] f32, out [512, 256, 64] f32.
Sharding: pure data parallel, 64 batches per NeuronCore across 8 cores.

Device algorithm per batch pair (all matmul operands bf16, fp32 PSUM accum):
  - host ships xT = x^T per batch ([E, T] layout, E on partitions).
  - qk^T = [Wq|Wk]^T @ xT        (one packed 128-wide stationary, 3 E-chunks,
                                  both batches of the pair in one N=512 MM)
  - v    = xT^T-chunk @ Wv       (x chunks stationary, Wv moving -> v in
                                  [t, h] layout directly; no PE transpose)
  - S^T  = k^T.T @ q^T           ([tk, tq] layout; lower-left T/4 block skipped)
  - P    = exp(0.125 * S^T)      (ScalarE; no max-subtraction needed, |s|<~45)
  - P   *= causal 0/1 tril       (only the two diagonal blocks)
  - out_aug[tq, 0:65] = sum_tk P[tk,tq] * [v|1][tk]  (col 64 = softmax denom)
  - out_aug -> SBUF bf16 -> HBM; the denominator divide happens on HOST.
The per-quad loop is software-pipelined: projections for quad q are emitted
before attention for quad q-1 so the PE never waits on the copy->shift chain.
"""

import numpy as np
import ml_dtypes

B, T, E, H = 512, 256, 384, 64
NCORES = 8
BPC = B // NCORES  # 64
P = 128
EC = E // P  # 3
HP1 = H + 1  # 65

_cache: dict = {}


def _install_ntff_hook():
    """Shim antenv.axon_hooks (absent in this image) so run_bass_kernel_spmd
    trace=True can capture NTFF profiles via the axon .so's C ABI."""
    import contextlib
    import ctypes
    import sys
    import types

    if "antenv.axon_hooks" in sys.modules:
        return
    so_path = "/opt/axon/libaxon_pjrt.so"
    lib = ctypes.CDLL(so_path)
    if not hasattr(lib, "axon_start_nrt_profile"):
        return
    lib.axon_start_nrt_profile.argtypes = [
        ctypes.POINTER(ctypes.c_int64),
        ctypes.c_size_t,
    ]
    lib.axon_start_nrt_profile.restype = ctypes.c_int64
    lib.axon_stop_nrt_profile.argtypes = [ctypes.c_char_p]
    lib.axon_stop_nrt_profile.restype = ctypes.c_int64

    @contextlib.contextmanager
    def _hook(output_dir, device_ids):
        import jax

        jax.devices()
        if device_ids:
            ids = (ctypes.c_int64 * len(device_ids))(*device_ids)
            rc = lib.axon_start_nrt_profile(ids, len(device_ids))
        else:
            rc = lib.axon_start_nrt_profile(None, 0)
        if rc != 0:
            raise RuntimeError(f"axon_start_nrt_profile rc={rc}")
        try:
            yield
        finally:
            n = lib.axon_stop_nrt_profile(str(output_dir).encode())
            if n < 0:
                raise RuntimeError(f"axon_stop_nrt_profile rc={n}")
            print(f"profile: {n} file(s) written to {output_dir}", file=sys.stderr)

    mod = types.ModuleType("antenv.axon_hooks")
    _state = {"hook": _hook}
    mod.get_axon_ntff_profile_hook = lambda: _state["hook"]
    mod.set_axon_ntff_profile_hook = lambda h: _state.__setitem__("hook", h)
    sys.modules["antenv.axon_hooks"] = mod


def _build_program(bpc):
    import concourse.bacc as bacc
    import concourse.mybir as mybir
    import concourse.tile as tile

    f32 = mybir.dt.float32
    bf16 = mybir.dt.bfloat16
    Exp = mybir.ActivationFunctionType.Exp
    Mult = mybir.AluOpType.mult

    nc = bacc.Bacc(
        "TRN2",
        target_bir_lowering=False,
        debug=False,
        enable_asserts=False,
        num_devices=NCORES,
    )
    xt_d = nc.dram_tensor("xt", [bpc, P, EC, T], bf16, kind="ExternalInput").ap()
    wqk_d = nc.dram_tensor("wqk", [P, EC, P], bf16, kind="ExternalInput").ap()
    wv_d = nc.dram_tensor("wv", [P, EC, H], bf16, kind="ExternalInput").ap()
    # 0/1 tril mask (tk <= tq) for the two diagonal 128x128 score blocks
    um_d = nc.dram_tensor("um", [P, P], bf16, kind="ExternalInput").ap()
    out_d = nc.dram_tensor("out", [bpc, T, HP1], bf16, kind="ExternalOutput").ap()

    Q = 4
    assert bpc % Q == 0
    nquads = bpc // Q

    with tile.TileContext(nc) as tc:
        with (
            tc.tile_pool(name="const", bufs=1) as constp,
            tc.tile_pool(name="xin", bufs=3) as xpool,
            tc.tile_pool(name="qksb", bufs=3) as qkpool,
            tc.tile_pool(name="psb", bufs=6) as ppool,
            tc.tile_pool(name="osb", bufs=3) as opool,
            tc.tile_pool(name="ps_qk", bufs=2, space="PSUM") as ps_qk,
            tc.tile_pool(name="ps_v", bufs=1, space="PSUM") as ps_v,
            tc.tile_pool(name="ps_s", bufs=3, space="PSUM") as ps_s,
            tc.tile_pool(name="ps_o", bufs=2, space="PSUM") as ps_o,
        ):
            wqk = constp.tile([P, EC, P], bf16)
            nc.sync.dma_start(wqk, wqk_d)
            wv = constp.tile([P, EC, H], bf16)
            nc.sync.dma_start(wv, wv_d)
            um = constp.tile([P, P], bf16)
            nc.sync.dma_start(um, um_d)
            # v staging [tk, s, j, h|1] with a persistent ones column at h=64
            # (manual rotation so the ones column survives across iterations)
            v_augs = []
            for i in range(8):
                va = constp.tile([P, 2, 2, HP1], bf16, name=f"vaug{i}")
                nc.vector.memset(va[:, :, :, H : H + 1], 1.0)
                v_augs.append(va)
            # k^T staging padded to 128 partitions with zero rows 64:128 so the
            # scores matmuls use full-width stationaries and stream q^T
            # directly from qk_sb (zero k rows null out the garbage rows)
            kabs = []
            for i in range(2):
                kt = constp.tile([P, Q, T], bf16, name=f"kab{i}")
                nc.vector.memset(kt[H:P], 0.0)
                kabs.append(kt)

            state = {}

            def emit_proj(qd):
                b0 = Q * qd
                xt = xpool.tile([P, Q, EC, T], bf16)
                nc.sync.dma_start(
                    xt, xt_d[b0 : b0 + Q].rearrange("s p c t -> p s c t")
                )
                qk_sb = qkpool.tile([P, Q, T], bf16)
                k_sb = kabs[qd % 2]
                for prl in range(Q // 2):
                    s0 = 2 * prl
                    pr = qd * (Q // 2) + prl
                    qk_ps = ps_qk.tile([P, 2, T], f32)
                    for c in range(EC):
                        nc.tensor.matmul(
                            qk_ps,
                            wqk[:, c, :],
                            xt[:, s0 : s0 + 2, c, :],
                            start=(c == 0),
                            stop=(c == EC - 1),
                        )
                    v_ps = ps_v.tile([P, 2, 2, H], f32)
                    for s in range(2):
                        for j in range(2):
                            for c in range(EC):
                                nc.tensor.matmul(
                                    v_ps[:, s, j, :],
                                    xt[:, s0 + s, c, j * P : (j + 1) * P],
                                    wv[:, c, :],
                                    start=(c == 0),
                                    stop=(c == EC - 1),
                                )
                    # PSUM -> SBUF bf16 casts
                    nc.vector.tensor_copy(qk_sb[:, s0 : s0 + 2, :], qk_ps)
                    v_aug = v_augs[pr % 8]
                    nc.vector.tensor_copy(v_aug[:, :, :, 0:H], v_ps)
                # k^T partitions 64:128 -> 0:64 (DMA shift), whole quad at once
                nc.sync.dma_start(k_sb[0:H], qk_sb[H:P])
                state[qd] = (qk_sb, k_sb)

            def emit_score(qd):
                qk_sb, k_sb = state[qd]
                p_sbs = []
                for prl in range(Q // 2):
                    s0 = 2 * prl
                    p_sb = ppool.tile([P, 2, EC, P], bf16)
                    for s in range(2):
                        s_ps = ps_s.tile([P, EC * P], f32, name="s_ps")
                        # S^T[tk 0:128, tq 0:256]
                        nc.tensor.matmul(
                            s_ps[:, 0:T],
                            k_sb[:, s0 + s, 0:P],
                            qk_sb[:, s0 + s, :],
                            start=True,
                            stop=True,
                        )
                        # S^T[tk 128:256, tq 128:256]
                        nc.tensor.matmul(
                            s_ps[:, T : 3 * P],
                            k_sb[:, s0 + s, P:T],
                            qk_sb[:, s0 + s, P:T],
                            start=True,
                            stop=True,
                        )
                        nc.scalar.activation(
                            p_sb[:, s], s_ps, Exp, scale=0.125
                        )
                    p_sbs.append(p_sb)
                state[qd] = p_sbs

            def emit_out(qd):
                b0 = Q * qd
                p_sbs = state.pop(qd)
                o_sb = opool.tile([P, Q, 2, HP1], bf16)
                for prl in range(Q // 2):
                    s0 = 2 * prl
                    pr = qd * (Q // 2) + prl
                    v_aug = v_augs[pr % 8]
                    p_sb = p_sbs[prl]

                    # multiplicative causal mask, diagonal blocks only
                    # (Vector and GpSimd take one each to balance load)
                    for blk, eng in ((0, nc.vector), (2, nc.gpsimd)):
                        eng.tensor_tensor(
                            p_sb[:, :, blk, :],
                            p_sb[:, :, blk, :],
                            um[:, None, :].to_broadcast([P, 2, P]),
                            Mult,
                        )

                    o_ps = ps_o.tile([P, 2, 2, HP1], f32)
                    for s in range(2):
                        nc.tensor.matmul(
                            o_ps[:, s, 0, :],
                            p_sb[:, s, 0, :],
                            v_aug[:, s, 0, :],
                            start=True,
                            stop=True,
                        )
                        nc.tensor.matmul(
                            o_ps[:, s, 1, :],
                            p_sb[:, s, 1, :],
                            v_aug[:, s, 0, :],
                            start=True,
                            stop=False,
                        )
                        nc.tensor.matmul(
                            o_ps[:, s, 1, :],
                            p_sb[:, s, 2, :],
                            v_aug[:, s, 1, :],
                            start=False,
                            stop=True,
                        )
                    nc.scalar.copy(o_sb[:, s0 : s0 + 2], o_ps)
                nc.gpsimd.dma_start(
                    out_d[b0 : b0 + Q].rearrange("s (j p) h -> p s j h", p=P),
                    o_sb,
                )

            for qd in range(nquads):
                emit_proj(qd)
                if qd >= 1:
                    emit_score(qd - 1)
                if qd >= 2:
                    emit_out(qd - 2)
            emit_score(nquads - 1)
            emit_out(nquads - 2)
            emit_out(nquads - 1)

    nc.compile()
    return nc


def _prep_inputs(x, Wq, Wk, Wv, bpc):
    bf = ml_dtypes.bfloat16
    nb = NCORES * bpc
    x = np.asarray(x, dtype=np.float32)[:nb]
    # [b, t, e] -> [b, p, c, t] with e = c*128 + p
    xt = np.ascontiguousarray(
        x.reshape(nb, T, EC, P).transpose(0, 3, 2, 1)
    ).astype(bf)
    wqk = np.concatenate(
        [np.asarray(Wq, np.float32), np.asarray(Wk, np.float32)], axis=1
    )  # [E, 128]
    wqk = np.ascontiguousarray(wqk.reshape(EC, P, P).transpose(1, 0, 2)).astype(bf)
    wv = np.ascontiguousarray(
        np.asarray(Wv, np.float32).reshape(EC, P, H).transpose(1, 0, 2)
    ).astype(bf)
    um = (np.arange(P)[:, None] <= np.arange(P)[None, :]).astype(np.float32).astype(bf)
    per_core = []
    for c in range(NCORES):
        per_core.append(
            {
                "xt": xt[c * bpc : (c + 1) * bpc],
                "wqk": wqk,
                "wv": wv,
                "um": um,
            }
        )
    return per_core


def kernel(x, Wq, Wk, Wv, _trace=False, _bpc=BPC):
    """Full inputs in, full output out. Shards batch dim over 8 NeuronCores."""
    from concourse import bass_utils

    if _trace:
        _install_ntff_hook()

    key = ("prog", _bpc)
    if key not in _cache:
        _cache[key] = _build_program(_bpc)
    nc = _cache[key]

    in_maps = _prep_inputs(x, Wq, Wk, Wv, _bpc)
    res = bass_utils.run_bass_kernel_spmd(
        nc, in_maps, core_ids=list(range(NCORES)), trace=_trace
    )
    _cache["last_result"] = res
    aug = np.concatenate(
        [np.asarray(r["out"]) for r in res.results], axis=0
    ).astype(np.float32)
    out = aug[:, :, 0:H] / aug[:, :, H : H + 1]
    return out


# revision 8
# speedup vs baseline: 1.0839x; 1.0709x over previous
"""Trainium2 Bass kernel: batched causal single-head self-attention.

Reference computation (per batch b):
    q = x @ Wq; k = x @ Wk; v = x @ Wv          # [T, H] each, contraction over E
    S = (q @ k^T) / sqrt(H)                     # [T, T]
    P = softmax(causal_mask(S), axis=-1)
    out = P @ v                                 # [T, H]

Shapes: x [512, 256, 384] f32, W* [384, 64] f32, out [512, 256, 64] f32.
Sharding: pure data parallel, 64 batches per NeuronCore across 8 cores.

Device algorithm per batch pair (all matmul operands bf16, fp32 PSUM accum):
  - host ships xT = x^T per batch ([E, T] layout, E on partitions).
  - qk^T = [Wq|Wk]^T @ xT        (one packed 128-wide stationary, 3 E-chunks,
                                  both batches of the pair in one N=512 MM)
  - v    = xT^T-chunk @ Wv       (x chunks stationary, Wv moving -> v in
                                  [t, h] layout directly; no PE transpose)
  - S^T  = k^T.T @ q^T           ([tk, tq] layout; lower-left T/4 block skipped)
  - P    = exp(0.125 * S^T)      (ScalarE; no max-subtraction needed, |s|<~45)
  - P   *= causal 0/1 tril       (only the two diagonal blocks)
  - out_aug[tq, 0:65] = sum_tk P[tk,tq] * [v|1][tk]  (col 64 = softmax denom)
  - out_aug -> SBUF bf16 -> HBM; the denominator divide happens on HOST.
The per-quad loop is software-pipelined: projections for quad q are emitted
before attention for quad q-1 so the PE never waits on the copy->shift chain.
"""

import numpy as np
import ml_dtypes

B, T, E, H = 512, 256, 384, 64
NCORES = 8
BPC = B // NCORES  # 64
P = 128
EC = E // P  # 3
HP1 = H + 1  # 65

_cache: dict = {}


def _install_ntff_hook():
    """Shim antenv.axon_hooks (absent in this image) so run_bass_kernel_spmd
    trace=True can capture NTFF profiles via the axon .so's C ABI."""
    import contextlib
    import ctypes
    import sys
    import types

    if "antenv.axon_hooks" in sys.modules:
        return
    so_path = "/opt/axon/libaxon_pjrt.so"
    lib = ctypes.CDLL(so_path)
    if not hasattr(lib, "axon_start_nrt_profile"):
        return
    lib.axon_start_nrt_profile.argtypes = [
        ctypes.POINTER(ctypes.c_int64),
        ctypes.c_size_t,
    ]
    lib.axon_start_nrt_profile.restype = ctypes.c_int64
    lib.axon_stop_nrt_profile.argtypes = [ctypes.c_char_p]
    lib.axon_stop_nrt_profile.restype = ctypes.c_int64

    @contextlib.contextmanager
    def _hook(output_dir, device_ids):
        import jax

        jax.devices()
        if device_ids:
            ids = (ctypes.c_int64 * len(device_ids))(*device_ids)
            rc = lib.axon_start_nrt_profile(ids, len(device_ids))
        else:
            rc = lib.axon_start_nrt_profile(None, 0)
        if rc != 0:
            raise RuntimeError(f"axon_start_nrt_profile rc={rc}")
        try:
            yield
        finally:
            n = lib.axon_stop_nrt_profile(str(output_dir).encode())
            if n < 0:
                raise RuntimeError(f"axon_stop_nrt_profile rc={n}")
            print(f"profile: {n} file(s) written to {output_dir}", file=sys.stderr)

    mod = types.ModuleType("antenv.axon_hooks")
    _state = {"hook": _hook}
    mod.get_axon_ntff_profile_hook = lambda: _state["hook"]
    mod.set_axon_ntff_profile_hook = lambda h: _state.__setitem__("hook", h)
    sys.modules["antenv.axon_hooks"] = mod


def _build_program(bpc):
    import concourse.bacc as bacc
    import concourse.mybir as mybir
    import concourse.tile as tile

    f32 = mybir.dt.float32
    bf16 = mybir.dt.bfloat16
    Exp = mybir.ActivationFunctionType.Exp
    Mult = mybir.AluOpType.mult

    nc = bacc.Bacc(
        "TRN2",
        target_bir_lowering=False,
        debug=False,
        enable_asserts=False,
        num_devices=NCORES,
    )
    xt_d = nc.dram_tensor("xt", [bpc, P, EC, T], bf16, kind="ExternalInput").ap()
    wqk_d = nc.dram_tensor("wqk", [P, EC, P], bf16, kind="ExternalInput").ap()
    wv_d = nc.dram_tensor("wv", [P, EC, H], bf16, kind="ExternalInput").ap()
    # 0/1 tril mask (tk <= tq) for the two diagonal 128x128 score blocks
    um_d = nc.dram_tensor("um", [P, P], bf16, kind="ExternalInput").ap()
    out_d = nc.dram_tensor("out", [bpc, T, HP1], bf16, kind="ExternalOutput").ap()

    Q = 4
    assert bpc % Q == 0
    nquads = bpc // Q

    with tile.TileContext(nc) as tc:
        with (
            tc.tile_pool(name="const", bufs=1) as constp,
            tc.tile_pool(name="xin", bufs=4) as xpool,
            tc.tile_pool(name="qksb", bufs=3) as qkpool,
            tc.tile_pool(name="psb", bufs=6) as ppool,
            tc.tile_pool(name="osb", bufs=3) as opool,
            tc.tile_pool(name="ps_qk", bufs=2, space="PSUM") as ps_qk,
            tc.tile_pool(name="ps_v", bufs=1, space="PSUM") as ps_v,
            tc.tile_pool(name="ps_s", bufs=3, space="PSUM") as ps_s,
            tc.tile_pool(name="ps_o", bufs=2, space="PSUM") as ps_o,
        ):
            wqk = constp.tile([P, EC, P], bf16)
            nc.sync.dma_start(wqk, wqk_d)
            wv = constp.tile([P, EC, H], bf16)
            nc.sync.dma_start(wv, wv_d)
            um = constp.tile([P, P], bf16)
            nc.sync.dma_start(um, um_d)
            # v staging [tk, s, j, h|1] with a persistent ones column at h=64
            # (manual rotation so the ones column survives across iterations)
            v_augs = []
            for i in range(8):
                va = constp.tile([P, 2, 2, HP1], bf16, name=f"vaug{i}")
                nc.vector.memset(va[:, :, :, H : H + 1], 1.0)
                v_augs.append(va)
            # k^T staging padded to 128 partitions with zero rows 64:128 so the
            # scores matmuls use full-width stationaries and stream q^T
            # directly from qk_sb (zero k rows null out the garbage rows)
            kabs = []
            for i in range(2):
                kt = constp.tile([P, Q, T], bf16, name=f"kab{i}")
                nc.vector.memset(kt[H:P], 0.0)
                kabs.append(kt)

            state = {}
            state_x = {}

            def emit_in(qd):
                b0 = Q * qd
                xt = xpool.tile([P, Q, EC, T], bf16)
                nc.sync.dma_start(
                    xt, xt_d[b0 : b0 + Q].rearrange("s p c t -> p s c t")
                )
                state_x[qd] = xt

            def emit_proj(qd):
                xt = state_x.pop(qd)
                qk_sb = qkpool.tile([P, Q, T], bf16)
                k_sb = kabs[qd % 2]
                for prl in range(Q // 2):
                    s0 = 2 * prl
                    pr = qd * (Q // 2) + prl
                    qk_ps = ps_qk.tile([P, 2, T], f32)
                    for c in range(EC):
                        nc.tensor.matmul(
                            qk_ps,
                            wqk[:, c, :],
                            xt[:, s0 : s0 + 2, c, :],
                            start=(c == 0),
                            stop=(c == EC - 1),
                        )
                    v_ps = ps_v.tile([P, 2, 2, H], f32)
                    for s in range(2):
                        for j in range(2):
                            for c in range(EC):
                                nc.tensor.matmul(
                                    v_ps[:, s, j, :],
                                    xt[:, s0 + s, c, j * P : (j + 1) * P],
                                    wv[:, c, :],
                                    start=(c == 0),
                                    stop=(c == EC - 1),
                                )
                    # PSUM -> SBUF bf16 casts
                    nc.vector.tensor_copy(qk_sb[:, s0 : s0 + 2, :], qk_ps)
                    v_aug = v_augs[pr % 8]
                    nc.vector.tensor_copy(v_aug[:, :, :, 0:H], v_ps)
                # k^T partitions 64:128 -> 0:64 (DMA shift), whole quad at once
                nc.sync.dma_start(k_sb[0:H], qk_sb[H:P])
                state[qd] = (qk_sb, k_sb)

            def emit_score(qd):
                qk_sb, k_sb = state[qd]
                p_sbs = []
                for prl in range(Q // 2):
                    s0 = 2 * prl
                    p_sb = ppool.tile([P, 2, EC, P], bf16)
                    for s in range(2):
                        s_ps = ps_s.tile([P, EC * P], f32, name="s_ps")
                        # S^T[tk 0:128, tq 0:256]
                        nc.tensor.matmul(
                            s_ps[:, 0:T],
                            k_sb[:, s0 + s, 0:P],
                            qk_sb[:, s0 + s, :],
                            start=True,
                            stop=True,
                        )
                        # S^T[tk 128:256, tq 128:256]
                        nc.tensor.matmul(
                            s_ps[:, T : 3 * P],
                            k_sb[:, s0 + s, P:T],
                            qk_sb[:, s0 + s, P:T],
                            start=True,
                            stop=True,
                        )
                        nc.scalar.activation(
                            p_sb[:, s], s_ps, Exp, scale=0.125
                        )
                    p_sbs.append(p_sb)
                state[qd] = p_sbs

            def emit_out(qd):
                b0 = Q * qd
                p_sbs = state.pop(qd)
                o_sb = opool.tile([P, Q, 2, HP1], bf16)
                for prl in range(Q // 2):
                    s0 = 2 * prl
                    pr = qd * (Q // 2) + prl
                    v_aug = v_augs[pr % 8]
                    p_sb = p_sbs[prl]

                    # multiplicative causal mask, diagonal blocks only
                    # (Vector and GpSimd take one each to balance load)
                    for blk, eng in ((0, nc.vector), (2, nc.gpsimd)):
                        eng.tensor_tensor(
                            p_sb[:, :, blk, :],
                            p_sb[:, :, blk, :],
                            um[:, None, :].to_broadcast([P, 2, P]),
                            Mult,
                        )

                    o_ps = ps_o.tile([P, 2, 2, HP1], f32)
                    for s in range(2):
                        nc.tensor.matmul(
                            o_ps[:, s, 0, :],
                            p_sb[:, s, 0, :],
                            v_aug[:, s, 0, :],
                            start=True,
                            stop=True,
                        )
                        nc.tensor.matmul(
                            o_ps[:, s, 1, :],
                            p_sb[:, s, 1, :],
                            v_aug[:, s, 0, :],
                            start=True,
                            stop=False,
                        )
                        nc.tensor.matmul(
                            o_ps[:, s, 1, :],
                            p_sb[:, s, 2, :],
                            v_aug[:, s, 1, :],
                            start=False,
                            stop=True,
                        )
                    nc.scalar.copy(o_sb[:, s0 : s0 + 2], o_ps)
                nc.gpsimd.dma_start(
                    out_d[b0 : b0 + Q].rearrange("s (j p) h -> p s j h", p=P),
                    o_sb,
                )

            PF = 3  # input-DMA prefetch depth in quads
            for qd in range(min(PF, nquads)):
                emit_in(qd)
            for qd in range(nquads):
                if qd + PF < nquads:
                    emit_in(qd + PF)
                emit_proj(qd)
                if qd >= 1:
                    emit_score(qd - 1)
                if qd >= 2:
                    emit_out(qd - 2)
            emit_score(nquads - 1)
            emit_out(nquads - 2)
            emit_out(nquads - 1)

    nc.compile()
    return nc


def _prep_inputs(x, Wq, Wk, Wv, bpc):
    bf = ml_dtypes.bfloat16
    nb = NCORES * bpc
    x = np.asarray(x, dtype=np.float32)[:nb]
    # [b, t, e] -> [b, p, c, t] with e = c*128 + p
    xt = np.ascontiguousarray(
        x.reshape(nb, T, EC, P).transpose(0, 3, 2, 1)
    ).astype(bf)
    wqk = np.concatenate(
        [np.asarray(Wq, np.float32), np.asarray(Wk, np.float32)], axis=1
    )  # [E, 128]
    wqk = np.ascontiguousarray(wqk.reshape(EC, P, P).transpose(1, 0, 2)).astype(bf)
    wv = np.ascontiguousarray(
        np.asarray(Wv, np.float32).reshape(EC, P, H).transpose(1, 0, 2)
    ).astype(bf)
    um = (np.arange(P)[:, None] <= np.arange(P)[None, :]).astype(np.float32).astype(bf)
    per_core = []
    for c in range(NCORES):
        per_core.append(
            {
                "xt": xt[c * bpc : (c + 1) * bpc],
                "wqk": wqk,
                "wv": wv,
                "um": um,
            }
        )
    return per_core


def kernel(x, Wq, Wk, Wv, _trace=False, _bpc=BPC):
    """Full inputs in, full output out. Shards batch dim over 8 NeuronCores."""
    from concourse import bass_utils

    if _trace:
        _install_ntff_hook()

    key = ("prog", _bpc)
    if key not in _cache:
        _cache[key] = _build_program(_bpc)
    nc = _cache[key]

    in_maps = _prep_inputs(x, Wq, Wk, Wv, _bpc)
    res = bass_utils.run_bass_kernel_spmd(
        nc, in_maps, core_ids=list(range(NCORES)), trace=_trace
    )
    _cache["last_result"] = res
    aug = np.concatenate(
        [np.asarray(r["out"]) for r in res.results], axis=0
    ).astype(np.float32)
    out = aug[:, :, 0:H] / aug[:, :, H : H + 1]
    return out
